# revision 14
# baseline (speedup 1.0000x reference)
"""Block-diagonal ZF equalizer (nn_BDEqualizer) as a Trainium2 Bass kernel.

Math: for every resource element (b, s, f) and UE u, solve the 8x8 complex
system H_u x_u = y_u where H_u[i, j] = h[b, 0, 8u+i, u, j, s, f] and
y_u[i] = y[b, 0, 8u+i, s, f].  Output x as [B, 1, 32, S, F, 2] (re/im last).

Strategy (data-parallel over the fft axis, per the sharding hint):
  - 8 cores, each owns a contiguous 128-subcarrier slice of F=1024.
  - Host pre-extracts the block-diagonal channel blocks (pure indexing) and
    ships per-core shards, plane-major so one DMA delivers one full matrix
    column-plane and the solve starts after ~2 plane loads.
  - On-chip layout: subcarriers on the 128 SBUF partitions, the other RE
    axes (u, b-pair, s) = 112 along the free dim.  Unpivoted complex
    Gaussian elimination + Jordan back-substitution on 9 augmented planes
    (8 matrix columns + rhs), re/im as separate fp32 tiles.
  - Three engines share the elimination.  With the sign-folded factor
    convention (gre = -Re F, fim = Im F, gim = -Im F where
    F = H[i,k]*conj(p)/|p|^2), every update is H_new = H_old + prodA +
    prodB, so per plane either (a) one engine does 4 products + 4
    accumulate-adds (DVE at 1.04 ns/elem or GpSimd/Pool at 1.98 ns/elem),
    or (b) an engine does only the 4 products and the TensorE accumulates
    H_old + prodA + prodB into PSUM via exact fp32 identity matmuls,
    with ScalarE copying the result back to SBUF.  A static 4-way planner
    (DVE/Pool/PE/Act) picks per-step assignments.
  - Software pipelining within a chunk: at step k DVE first updates pivot
    plane k+1, then immediately computes step k+1's pivot reciprocal
    (1-cpe approximate reciprocal) and factors into double-buffered factor
    tiles, so Pool's step-k+1 work is never factor-starved.
  - Software pipelining across chunks: chunk 0's back-substitution runs
    DVE-only, its steps interleaved on the DVE queue with chunk 1's
    forward steps, while Pool absorbs a biased (larger) share of chunk
    1's early forward work.
"""

import math
import os

import numpy as np

import concourse.bacc as bacc
import concourse.mybir as mybir
from concourse.bass_utils import run_bass_kernel_spmd
from concourse.masks import make_identity
from concourse.tile import TileContext

B, NRX, NR, U, A, S, F = 4, 1, 32, 4, 8, 14, 1024
NCORES = 8
FS = F // NCORES        # 128 subcarriers per core
NB = 2                  # batch entries per chunk
NCH = B // NB           # chunks per core
M = U * NB * S          # 112 RE columns per chunk (u, b, s)
NP = 9                  # augmented planes: 8 matrix columns + rhs
F32 = mybir.dt.float32
AL = mybir.AluOpType

LAST_RESULTS = None     # BassKernelResults of the most recent run (for test.py)

# --- static DVE/Pool/PE work-splitting planner ------------------------------
U_D = 112 * (1e9 / 0.96e9)          # DVE ns per M-unit (112 elems)
U_P = 112 * (1e9 / 1.2e9) / 0.42    # Pool ns per M-unit
U_PE = 112 * 2.2                    # PE fp32 matmul ns per M-unit (pstate avg)
U_A = 112 * (1e9 / 1.2e9)           # Act ns per M-unit
O_D = 60.0                          # DVE per-instruction busy adder
O_P = 95.0                          # Pool q7 launch per instruction
O_PE = 50.0                         # PE per-matmul adder
O_A = 185.0                         # Act per-instruction busy adder
W_D = 2                             # DVE plane-group width
W_P = 2                             # Pool plane-group width


def _subs(n):
    """Row subgroups of <=4 rows (PSUM bank is 512 fp32)."""
    if n <= 4:
        return [n]
    return [4, n - 4] if n != 6 else [3, 3]


def bwd_step_cost(j):
    """Modeled DVE time of a DVE-only Jordan back step with j rows."""
    return (6 + 8 * j) * U_D + (4 + 8 * (1 if j else 0)) * O_D


def plan_fwd(k, bias=0.0, use_pe=True):
    """Assignment for elimination step k.

    Returns (e, d_full, q_full, d_prod): planes k+2..8 are split as
    `e` PE-accumulated planes (taken from the top, products on DVE/Pool),
    `d_full` DVE-full planes (lowest), `q_full` Pool-full planes; of the
    e PE planes' product jobs, `d_prod` go to DVE and the rest to Pool.
    Plane k+1 is always DVE-full (it gates step k+1's factors).
    """
    n = 7 - k
    R = 7 - k  # planes k+2..8
    best, best_t = (0, R, 0, 0), float("inf")
    emax = R if use_pe else 0
    for e in range(0, emax + 1):
        nsub = len(_subs(n))
        t_pe = e * 2 * (3 * n * U_PE / 3 + nsub * 3 * O_PE)
        t_pe = e * (6 * n * U_PE + nsub * 6 * O_PE)
        t_act = e * (2 * n * U_A + nsub * 2 * O_A)
        for d_full in range(0, R - e + 1):
            q_full = R - e - d_full
            for d_prod in range(0, e + 1):
                q_prod = e - d_prod
                du = (6 + 7 * n + 8 * n) + 8 * n * d_full + 4 * n * d_prod
                pu = 8 * n * q_full + 4 * n * q_prod
                di = 19 + 8 * math.ceil(d_full / W_D) + 4 * d_prod
                pi = 8 * math.ceil(q_full / W_P) + 4 * q_prod
                t_d = du * U_D + di * O_D + bias
                t_p = pu * U_P + pi * O_P + (140.0 if pu else 0.0)
                t = max(t_d, t_p, t_pe, t_act)
                if t < best_t:
                    best_t, best = t, (e, d_full, q_full, d_prod)
    return best


def plan_bwd(k):
    """Pool rows (from the bottom) for the Jordan back pass at step k."""
    best, best_t = 0, float("inf")
    for m in range(0, k):
        t_d = (6 + 8 * (k - m)) * U_D + 14 * O_D
        t_p = 8 * m * U_P + (8 * O_P + 140.0 if m else 0.0)
        t = max(t_d, t_p)
        if t < best_t:
            best_t, best = t, m
    return best


def _off(j, i):
    """Free-dim offset of (plane j, row i) inside an H supertile."""
    return (j * A + i) * M


def _build():
    nc = bacc.Bacc(trn_type="TRN2")

    # Host-prepped layouts, plane-major: hd[j, u, b, s, i, f],
    # yd[u, b, s, i, f], out[i, u, b, s, f, c].  (i = row, j = column.)
    hdre = nc.dram_tensor("hd_re", [A, U, B, S, A, FS], F32, kind="ExternalInput")
    hdim = nc.dram_tensor("hd_im", [A, U, B, S, A, FS], F32, kind="ExternalInput")
    ydre = nc.dram_tensor("yd_re", [U, B, S, A, FS], F32, kind="ExternalInput")
    ydim = nc.dram_tensor("yd_im", [U, B, S, A, FS], F32, kind="ExternalInput")
    out = nc.dram_tensor("out", [A, U, B, S, FS, 2], F32, kind="ExternalOutput")

    with TileContext(nc) as tc:
        with (
            tc.tile_pool(name="consts", bufs=1) as consts,
            tc.tile_pool(name="supers", bufs=2) as supers,
            tc.tile_pool(name="work", bufs=1) as work,
            tc.tile_pool(name="scr", bufs=4) as scr,
            tc.tile_pool(name="stg", bufs=2) as stg,
            tc.tile_pool(name="stgo", bufs=1) as stgo,
            tc.tile_pool(name="psin", bufs=2, space="PSUM") as psin,
            tc.tile_pool(name="pso", bufs=2, space="PSUM") as pso_pool,
            tc.tile_pool(name="psacc", bufs=4, space="PSUM") as psacc_pool,
        ):
            ident = consts.tile([128, 128], F32)
            make_identity(nc, ident)

            # INV holds pivot reciprocals per chunk parity at base 0 /
            # 2*A*M (qr block | qi block, + A*M view padding).
            INV = work.tile([128, 5 * A * M], F32, tag="INV")
            GRe0 = work.tile([128, 7 * M], F32, tag="GRe0")
            GRe1 = work.tile([128, 7 * M], F32, tag="GRe1")
            FIm0 = work.tile([128, 7 * M], F32, tag="FIm0")
            FIm1 = work.tile([128, 7 * M], F32, tag="FIm1")
            GIm0 = work.tile([128, 7 * M], F32, tag="GIm0")
            GIm1 = work.tile([128, 7 * M], F32, tag="GIm1")
            GRe = (GRe0, GRe1)
            FIm = (FIm0, FIm1)
            GIm = (GIm0, GIm1)
            TD = work.tile([128, M], F32, tag="TD")
            TU = work.tile([128, M], F32, tag="TU")
            TR = work.tile([128, M], F32, tag="TR")

            def stile():
                t = scr.tile([128, 2 * 7 * M], F32, tag="scr", name="scrt")
                return t

            def make_chunk(ci):
                HRe = supers.tile([128, (NP + 1) * A * M], F32, tag="HRe")
                HIm = supers.tile([128, (NP + 1) * A * M], F32, tag="HIm")
                return {
                    "ci": ci,
                    "b0": ci * NB,
                    "HRe": HRe,
                    "HIm": HIm,
                    "ibase": (ci % 2) * 2 * A * M,
                }

            def row(T, j, i):
                return T[:, _off(j, i) : _off(j, i) + M]

            def rows3(T, j, i0, n):
                base = _off(j, i0)
                return T[:, base : base + n * M].rearrange("p (r c) -> p r c", r=n)

            def bc(ap, n):
                return ap[:, None, :].broadcast_to([128, n, M])

            def emit_load(C):
                # Plane order matches consumption: DVE needs 0,1,2 first,
                # Pool's first groups touch the top planes.
                b0 = C["b0"]
                for j in (0, 1, 6, 7, 2, 8, 3, 4, 5):
                    for comp in range(2):
                        if j == 8:
                            src = (ydre, ydim)[comp][:, b0 : b0 + NB]
                        else:
                            src = (hdre, hdim)[comp][j, :, b0 : b0 + NB]
                        for ig in range(2):
                            stage = stg.tile([M, 4 * FS], F32, tag="stage")
                            nc.sync.dma_start(
                                stage, src[:, :, :, 4 * ig : 4 * ig + 4]
                            )
                            ps = psin.tile([128, 4 * M], F32, tag="psin")
                            for q in range(4):
                                nc.tensor.transpose(
                                    ps[:, q * M : (q + 1) * M],
                                    stage[:, q * FS : (q + 1) * FS],
                                    ident[:M, :M],
                                )
                            base = _off(j, ig * 4)
                            nc.scalar.copy(
                                C[("HRe", "HIm")[comp]][:, base : base + 4 * M],
                                ps,
                            )

            def inv_pair(C, k, n=None):
                # (ir_k, ii_k) as [128, 2, M]; broadcast over n rows if set
                b = C["ibase"]
                v = INV[:, b + k * M : b + k * M + 2 * A * M].rearrange(
                    "p (j c) -> p j c", j=2
                )[:, :, :M]
                if n is None:
                    return v
                return v[:, :, None, :].broadcast_to([128, 2, n, M])

            def pivot_chain(C, k):
                # ir_k + i*ii_k = conj(p)/|p|^2 for pivot p of step k
                b = C["ibase"]
                a = row(C["HRe"], k, k)
                b_ = row(C["HIm"], k, k)
                nc.vector.tensor_mul(TD, a, a)
                nc.vector.tensor_mul(TU, b_, b_)
                nc.vector.tensor_add(TD, TD, TU)
                nc.vector.reciprocal_approx_fast(TR, TD)
                irk = INV[:, b + k * M : b + (k + 1) * M]
                iik = INV[:, b + (A + k) * M : b + (A + k + 1) * M]
                nc.vector.tensor_mul(irk, a, TR)
                nc.vector.tensor_mul(iik, b_, TR)

            def factors(C, k):
                # Sign-folded factors for i in k+1..7:
                # gre = -(cr*ir + ci*ii), fim = ci*ir - cr*ii, gim = -fim
                n = A - 1 - k
                car = rows3(C["HRe"], k, k + 1, n)
                cai = rows3(C["HIm"], k, k + 1, n)
                car4 = car[:, None, :, :].broadcast_to([128, 2, n, M])
                cai4 = cai[:, None, :, :].broadcast_to([128, 2, n, M])
                sa, sb = stile(), stile()
                p1 = sa[:, : 2 * n * M].rearrange("p (j c) -> p j c", j=2)
                p2 = sb[:, : 2 * n * M].rearrange("p (j c) -> p j c", j=2)
                nc.vector.tensor_mul(p1, car4, inv_pair(C, k, n))
                nc.vector.tensor_mul(p2, cai4, inv_pair(C, k, n))
                gre = GRe[k % 2][:, : n * M]
                fim = FIm[k % 2][:, : n * M]
                gim = GIm[k % 2][:, : n * M]
                nc.vector.scalar_tensor_tensor(
                    gre, sa[:, : n * M], -1.0, sb[:, n * M : 2 * n * M],
                    AL.mult, AL.subtract,
                )
                nc.vector.tensor_sub(fim, sb[:, : n * M], sa[:, n * M : 2 * n * M])
                nc.vector.tensor_sub(gim, sa[:, n * M : 2 * n * M], sb[:, : n * M])

            def elim_group(C, eng, k, j0, w, i0, nr):
                """Full update: eliminate col k from planes [j0, j0+w),
                rows [i0, i0+nr):  hr += gre*Br + fim*Bi,
                                   hi += gre*Bi + gim*Br."""
                HRe_, HIm_ = C["HRe"], C["HIm"]
                sa, sb = stile(), stile()

                def wrows(T):
                    base = _off(j0, i0)
                    return T[:, base : base + w * A * M].rearrange(
                        "p (w c) -> p w c", w=w
                    )[:, :, : nr * M]

                def wrow_b(T):
                    base = _off(j0, k)
                    v = T[:, base : base + w * A * M].rearrange(
                        "p (w c) -> p w c", w=w
                    )[:, :, :M]
                    return v[:, :, None, :].broadcast_to([128, w, nr, M])

                def fw(Ft):
                    o = (i0 - k - 1) * M
                    v = Ft[:, o : o + nr * M].rearrange("p (r c) -> p r c", r=nr)
                    return v[:, None, :, :].broadcast_to([128, w, nr, M])

                hr, hi = wrows(HRe_), wrows(HIm_)
                Br, Bi = wrow_b(HRe_), wrow_b(HIm_)
                grew = fw(GRe[k % 2])
                fimw = fw(FIm[k % 2])
                gimw = fw(GIm[k % 2])
                sz = w * nr * M
                A4 = sa[:, :sz].rearrange("p (w r c) -> p w r c", w=w, r=nr)
                A3 = sa[:, :sz].rearrange("p (w c) -> p w c", w=w)
                B4 = sb[:, :sz].rearrange("p (w r c) -> p w r c", w=w, r=nr)
                B3 = sb[:, :sz].rearrange("p (w c) -> p w c", w=w)
                eng.tensor_mul(A4, grew, Br)
                eng.tensor_mul(B4, grew, Bi)
                eng.tensor_add(hr, hr, A3)
                eng.tensor_add(hi, hi, B3)
                eng.tensor_mul(A4, fimw, Bi)
                eng.tensor_mul(B4, gimw, Br)
                eng.tensor_add(hr, hr, A3)
                eng.tensor_add(hi, hi, B3)

            def emit_groups(C, eng, k, planes, i0, nr, wmax):
                js = list(planes)
                while js:
                    w = 1
                    while w < wmax and w < len(js) and js[w] == js[0] + w:
                        w += 1
                    elim_group(C, eng, k, js[0], w, i0, nr)
                    js = js[w:]

            def elim_plane_pe(C, eng, k, j):
                """PE-accumulated update of plane j at step k: `eng` does
                the 4 products, TensorE sums H_old + prodA + prodB into
                PSUM (exact fp32 matmuls), ScalarE copies back."""
                n = A - 1 - k
                i0 = k + 1
                HRe_, HIm_ = C["HRe"], C["HIm"]
                gre = rows3(GRe[k % 2], 0, 0, n) if False else None
                fv = lambda Ft: Ft[:, : n * M].rearrange("p (r c) -> p r c", r=n)
                BrB = bc(row(HRe_, j, k), n)
                BiB = bc(row(HIm_, j, k), n)
                sre, sim = stile(), stile()
                sre3 = lambda h: sre[:, h * n * M : (h + 1) * n * M].rearrange(
                    "p (r c) -> p r c", r=n
                )
                sim3 = lambda h: sim[:, h * n * M : (h + 1) * n * M].rearrange(
                    "p (r c) -> p r c", r=n
                )
                # hr += gre*Br + fim*Bi ; hi += gre*Bi + gim*Br
                eng.tensor_mul(sre3(0), fv(GRe[k % 2]), BrB)
                eng.tensor_mul(sre3(1), fv(FIm[k % 2]), BiB)
                eng.tensor_mul(sim3(0), fv(GRe[k % 2]), BiB)
                eng.tensor_mul(sim3(1), fv(GIm[k % 2]), BrB)
                for comp, sc in ((0, sre), (1, sim)):
                    T = (HRe_, HIm_)[comp]
                    r0 = 0
                    for nr in _subs(n):
                        acc = psacc_pool.tile([128, 4 * M], F32, tag="acc")
                        av = acc[:, : nr * M]
                        hsl = T[:, _off(j, i0 + r0) : _off(j, i0 + r0) + nr * M]
                        pa = sc[:, r0 * M : (r0 + nr) * M]
                        pb = sc[:, (n + r0) * M : (n + r0 + nr) * M]
                        nc.tensor.matmul(av, ident, hsl, start=True, stop=False)
                        nc.tensor.matmul(av, ident, pa, start=False, stop=False)
                        nc.tensor.matmul(av, ident, pb, start=False, stop=True)
                        nc.scalar.copy(hsl, av)
                        r0 += nr

            def emit_prep0(C):
                pivot_chain(C, 0)
                factors(C, 0)

            def fwd_step(C, k, dve=True, pool=True, bias=0.0, use_pe=True):
                """Elimination step k: plane k+1 DVE-full, then next step's
                prep, then the planned PE/full/product assignments."""
                n = A - 1 - k
                e, d_full, q_full, d_prod = plan_fwd(k, bias, use_pe)
                pe_planes = list(range(9 - e, 9))
                lo = k + 2
                dve_planes = list(range(lo, lo + d_full))
                pool_planes = list(range(lo + d_full, lo + d_full + q_full))
                dve_prod = pe_planes[:d_prod]
                pool_prod = pe_planes[d_prod:]
                if dve:
                    elim_group(C, nc.vector, k, k + 1, 1, k + 1, n)
                    pivot_chain(C, k + 1)
                    if k + 1 < A - 1:
                        factors(C, k + 1)
                if pool:
                    for j in pool_prod:
                        elim_plane_pe(C, nc.gpsimd, k, j)
                    if pool_planes:
                        emit_groups(C, nc.gpsimd, k, pool_planes, k + 1, n, W_P)
                if dve:
                    for j in dve_prod:
                        elim_plane_pe(C, nc.vector, k, j)
                    if dve_planes:
                        emit_groups(C, nc.vector, k, dve_planes, k + 1, n, W_D)

            def bwd_step(C, k, use_pool):
                """Jordan back step k: x_k = y_k*conj(p)/|p|^2, store it,
                then clear column k above the diagonal."""
                HRe_, HIm_ = C["HRe"], C["HIm"]
                yr = row(HRe_, 8, k)
                yi = row(HIm_, 8, k)
                sa, sb = stile(), stile()
                p1 = sa[:, : 2 * M].rearrange("p (j c) -> p j c", j=2)
                p2 = sb[:, : 2 * M].rearrange("p (j c) -> p j c", j=2)
                yr2 = yr[:, None, :].broadcast_to([128, 2, M])
                yi2 = yi[:, None, :].broadcast_to([128, 2, M])
                nc.vector.tensor_mul(p1, yr2, inv_pair(C, k))
                nc.vector.tensor_mul(p2, yi2, inv_pair(C, k))
                # xr = yr*ir + yi*ii, xi = yi*ir - yr*ii
                nc.vector.tensor_add(yr, sa[:, :M], sb[:, M : 2 * M])
                nc.vector.tensor_sub(yi, sb[:, :M], sa[:, M : 2 * M])
                # x_k is final now -- store it while the back pass continues.
                so = stgo.tile([M, 2 * FS], F32, tag="so")
                so3 = so.rearrange("p (f c) -> p f c", c=2)
                for comp in range(2):
                    po = pso_pool.tile([M, FS], F32, tag="pso")
                    nc.tensor.transpose(
                        po, row(C[("HRe", "HIm")[comp]], 8, k), ident[:128, :128]
                    )
                    nc.scalar.copy(so3[:, :, comp], po)
                nc.sync.dma_start(out[k, :, C["b0"] : C["b0"] + NB], so)
                if k == 0:
                    return
                m = plan_bwd(k) if use_pool else 0

                def yupd(eng, r0, nr):
                    # y_i -= H[i,k]*x_k for rows [r0, r0+nr)
                    qsa, qsb = stile(), stile()
                    cr = rows3(HRe_, k, r0, nr)
                    ci_ = rows3(HIm_, k, r0, nr)
                    xrB = bc(yr, nr)
                    xiB = bc(yi, nr)
                    qa = qsa[:, : nr * M].rearrange("p (r c) -> p r c", r=nr)
                    qc = qsa[:, 7 * M : (7 + nr) * M].rearrange(
                        "p (r c) -> p r c", r=nr
                    )
                    qb = qsb[:, : nr * M].rearrange("p (r c) -> p r c", r=nr)
                    qd = qsb[:, 7 * M : (7 + nr) * M].rearrange(
                        "p (r c) -> p r c", r=nr
                    )
                    ytr = rows3(HRe_, 8, r0, nr)
                    yti = rows3(HIm_, 8, r0, nr)
                    eng.tensor_mul(qa, cr, xrB)
                    eng.tensor_mul(qc, cr, xiB)
                    eng.tensor_sub(ytr, ytr, qa)
                    eng.tensor_sub(yti, yti, qc)
                    eng.tensor_mul(qb, ci_, xiB)
                    eng.tensor_mul(qd, ci_, xrB)
                    eng.tensor_add(ytr, ytr, qb)
                    eng.tensor_sub(yti, yti, qd)

                if m > 0:
                    yupd(nc.gpsimd, 0, m)
                if k - m > 0:
                    yupd(nc.vector, m, k - m)

            # ---------------- emission schedule ----------------
            c0 = make_chunk(0)
            c1 = make_chunk(1)

            emit_load(c0)
            emit_prep0(c0)
            for k in range(A - 1):
                # step 0 of chunk 0: no PE accumulation (its Act copy-backs
                # would queue behind the chunk's load copies)
                fwd_step(c0, k, use_pe=(k > 0))
            emit_load(c1)
            # chunk 1 prep + Pool's step-0 share start while DVE runs
            # chunk 0's back pass, interleaved with chunk 1's DVE forward
            # steps; the planner bias hands Pool the slack.
            emit_prep0(c1)
            fwd_step(c1, 0, dve=False, pool=True, bias=bwd_step_cost(7))
            bwd_step(c0, 7, use_pool=False)
            fwd_step(c1, 0, dve=True, pool=False, bias=bwd_step_cost(7))
            for k in range(1, A - 1):
                j = 7 - k
                bwd_step(c0, j, use_pool=False)
                fwd_step(c1, k, bias=bwd_step_cost(j))
            bwd_step(c0, 0, use_pool=False)
            for k in range(A - 1, -1, -1):
                bwd_step(c1, k, use_pool=True)

    nc.finalize()
    return nc


_NC_CACHE = None


def _get_nc():
    global _NC_CACHE
    if _NC_CACHE is None:
        _NC_CACHE = _build()
    return _NC_CACHE


def _prep_core(y_re, y_im, h_re, h_im, c):
    """Host-side shard prep for core c: f-slice + block-diagonal extraction."""
    fsl = slice(c * FS, (c + 1) * FS)
    ue = np.arange(U)
    maps = {}
    for name, h in (("hd_re", h_re), ("hd_im", h_im)):
        h6 = h[:, 0, :, :, :, :, fsl].reshape(B, U, A, U, A, S, FS)
        hd = h6[:, ue, :, ue]              # [u, b, i, j, s, f]
        maps[name] = np.ascontiguousarray(
            hd.transpose(3, 0, 1, 4, 2, 5), dtype=np.float32
        )                                   # [j, u, b, s, i, f]
    for name, y in (("yd_re", y_re), ("yd_im", y_im)):
        y5 = y[:, 0, :, :, fsl].reshape(B, U, A, S, FS)   # [b, u, i, s, f]
        maps[name] = np.ascontiguousarray(
            y5.transpose(1, 0, 3, 2, 4), dtype=np.float32
        )                                   # [u, b, s, i, f]
    return maps


def kernel(y_re, y_im, h_re, h_im, **_ignored):
    global LAST_RESULTS
    y_re = np.asarray(y_re, dtype=np.float32)
    y_im = np.asarray(y_im, dtype=np.float32)
    h_re = np.asarray(h_re, dtype=np.float32)
    h_im = np.asarray(h_im, dtype=np.float32)

    nc = _get_nc()
    in_maps = [_prep_core(y_re, y_im, h_re, h_im, c) for c in range(NCORES)]
    trace = bool(int(os.environ.get("BD_TRACE", "0")))
    res = run_bass_kernel_spmd(
        nc, in_maps, core_ids=list(range(NCORES)), trace=trace
    )
    LAST_RESULTS = res
    outs = []
    for r in res.results:
        o = r["out"]                              # [i, u, b, s, f, c]
        o = o.transpose(2, 1, 0, 3, 4, 5)         # [b, u, i, s, f, c]
        outs.append(o.reshape(B, NR, S, FS, 2))
    full = np.concatenate(outs, axis=3)           # [B, NR, S, F, 2]
    return np.ascontiguousarray(full[:, None])    # [B, 1, NR, S, F, 2]


# revision 15
# speedup vs baseline: 1.3296x; 1.3296x over previous
"""Block-diagonal ZF equalizer (nn_BDEqualizer) as a Trainium2 Bass kernel.

Math: for every resource element (b, s, f) and UE u, solve the 8x8 complex
system H_u x_u = y_u where H_u[i, j] = h[b, 0, 8u+i, u, j, s, f] and
y_u[i] = y[b, 0, 8u+i, s, f].  Output x as [B, 1, 32, S, F, 2] (re/im last).

Strategy (data-parallel over the fft axis, per the sharding hint):
  - 8 cores, each owns a contiguous 128-subcarrier slice of F=1024.
  - Host pre-extracts the block-diagonal channel blocks (pure indexing) and
    ships per-core shards, plane-major so one DMA delivers one full matrix
    column-plane and the solve starts after ~2 plane loads.
  - On-chip layout: subcarriers on the 128 SBUF partitions, the other RE
    axes (u, b-pair, s) = 112 along the free dim.  Unpivoted complex
    Gaussian elimination + Jordan back-substitution on 9 augmented planes
    (8 matrix columns + rhs), re/im as separate fp32 tiles.
  - The elimination work (4 products + 4 accumulates per complex MAC, all
    plain tensor_tensor ops thanks to the unnegated factor convention
    F = H[i,k]*conj(p)/|p|^2) is split between the Vector engine and the
    GpSimd (Pool) engine, which run concurrently: a static planner assigns
    whole planes (and boundary-plane row ranges) per elimination step to
    balance DVE (1.04 ns/elem) against Pool (1.98 ns/elem).
  - Software pipelining within a chunk: at step k DVE first updates pivot
    plane k+1, then immediately computes step k+1's pivot reciprocal
    (1-cpe approximate reciprocal) and factors into double-buffered factor
    tiles, so Pool's step-k+1 work is never factor-starved.
  - Software pipelining across chunks: chunk 0's back-substitution runs
    DVE-only, its steps interleaved on the DVE queue with chunk 1's
    forward steps, while Pool absorbs a biased (larger) share of chunk
    1's early forward work.  This hides both the serial x-chain of the
    back pass and Pool's idle time there.
"""

import math
import os

import numpy as np

import concourse.bacc as bacc
import concourse.mybir as mybir
from concourse.bass_utils import run_bass_kernel_spmd
from concourse.masks import make_identity
from concourse.tile import TileContext

B, NRX, NR, U, A, S, F = 4, 1, 32, 4, 8, 14, 1024
NCORES = 8
FS = F // NCORES        # 128 subcarriers per core
NB = 2                  # batch entries per chunk
NCH = B // NB           # chunks per core
M = U * NB * S          # 112 RE columns per chunk (u, b, s)
NP = 9                  # augmented planes: 8 matrix columns + rhs
F32 = mybir.dt.float32
AL = mybir.AluOpType

LAST_RESULTS = None     # BassKernelResults of the most recent run (for test.py)

# --- static DVE/Pool work-splitting planner ---------------------------------
U_D = 112 * (1e9 / 0.96e9)          # DVE ns per M-unit (112 elems)
U_P = 112 * (1e9 / 1.2e9) / 0.42    # Pool ns per M-unit
O_D = 60.0                          # DVE per-instruction busy adder
O_P = 95.0                          # Pool q7 launch per instruction
W_D = 2                             # DVE plane-group width
W_P = 2                             # Pool plane-group width


def bwd_step_cost(j):
    """Modeled DVE time of a DVE-only Jordan back step with j rows."""
    return (6 + 8 * j) * U_D + (4 + 8 * (1 if j else 0)) * O_D


def plan_fwd(k, bias=0.0):
    """Pool assignment for elimination step k: (n_full_planes_from_top, rows).

    Pool takes full planes j in (8-npl, 8] plus the top `rs` rows of plane
    8-npl; plane k+1 always stays fully on DVE (it gates step k+1's
    factors).  Chosen to balance modeled engine busy times; `bias` is
    extra modeled DVE time (e.g. an interleaved back-sub step of the
    previous chunk) that Pool should absorb.
    """
    n = 7 - k
    nplanes = 8 - k
    best, best_t = (0, 0), float("inf")
    for npl in range(0, nplanes):
        j_b = 8 - npl
        max_rs = n - 1 if j_b >= k + 2 else 0
        for rs in range(0, max_rs + 1):
            pool_u = 8 * (n * npl + rs)
            dve_u = (6 + 6 * n) + 8 * n * nplanes - pool_u
            dve_full = nplanes - npl - (1 if rs > 0 else 0)
            dve_i = 10 + 8 * math.ceil(dve_full / W_D) + (8 if rs > 0 else 0)
            pool_i = 8 * math.ceil(npl / W_P) + (8 if rs > 0 else 0)
            t_d = dve_u * U_D + dve_i * O_D + bias
            t_p = pool_u * U_P + pool_i * O_P + (140.0 if pool_u else 0.0)
            t = max(t_d, t_p)
            if t < best_t:
                best_t, best = t, (npl, rs)
    return best


def plan_bwd(k):
    """Pool rows (from the bottom) for the Jordan back pass at step k."""
    best, best_t = 0, float("inf")
    for m in range(0, k):
        t_d = (6 + 8 * (k - m)) * U_D + 14 * O_D
        t_p = 8 * m * U_P + (8 * O_P + 140.0 if m else 0.0)
        t = max(t_d, t_p)
        if t < best_t:
            best_t, best = t, m
    return best


def _off(j, i):
    """Free-dim offset of (plane j, row i) inside an H supertile."""
    return (j * A + i) * M


def _build():
    nc = bacc.Bacc(trn_type="TRN2")

    # Host-prepped layouts, plane-major: hd[j, u, b, s, i, f],
    # yd[u, b, s, i, f], out[i, u, b, s, f, c].  (i = row, j = column.)
    hdre = nc.dram_tensor("hd_re", [A, U, B, S, A, FS], F32, kind="ExternalInput")
    hdim = nc.dram_tensor("hd_im", [A, U, B, S, A, FS], F32, kind="ExternalInput")
    ydre = nc.dram_tensor("yd_re", [U, B, S, A, FS], F32, kind="ExternalInput")
    ydim = nc.dram_tensor("yd_im", [U, B, S, A, FS], F32, kind="ExternalInput")
    out = nc.dram_tensor("out", [A, U, B, S, FS, 2], F32, kind="ExternalOutput")

    with TileContext(nc) as tc:
        with (
            tc.tile_pool(name="consts", bufs=1) as consts,
            tc.tile_pool(name="supers", bufs=2) as supers,
            tc.tile_pool(name="work", bufs=1) as work,
            tc.tile_pool(name="stg", bufs=2) as stg,
            tc.tile_pool(name="stgo", bufs=2) as stgo,
            tc.tile_pool(name="psin", bufs=3, space="PSUM") as psin,
            tc.tile_pool(name="pso", bufs=2, space="PSUM") as pso_pool,
        ):
            ident = consts.tile([128, 128], F32)
            make_identity(nc, ident)

            # Shared work tiles (single-buffered; engines are in-order so
            # same-engine reuse is safe, and the two engines use disjoint
            # scratch).  INV holds pivot reciprocals per chunk parity at
            # base 0 / 2*A*M (qr block | qi block, + A*M view padding).
            INV = work.tile([128, 5 * A * M], F32, tag="INV")
            FRe0 = work.tile([128, 7 * M], F32, tag="FRe0")
            FRe1 = work.tile([128, 7 * M], F32, tag="FRe1")
            FIm0 = work.tile([128, 7 * M], F32, tag="FIm0")
            FIm1 = work.tile([128, 7 * M], F32, tag="FIm1")
            FRe = (FRe0, FRe1)
            FIm = (FIm0, FIm1)
            PAs = work.tile([128, W_D * 7 * M], F32, tag="PAs")
            PBs = work.tile([128, W_D * 7 * M], F32, tag="PBs")
            PPa = work.tile([128, W_P * 7 * M], F32, tag="PPa")
            PPb = work.tile([128, W_P * 7 * M], F32, tag="PPb")
            TD = work.tile([128, M], F32, tag="TD")
            TU = work.tile([128, M], F32, tag="TU")
            TR = work.tile([128, M], F32, tag="TR")

            def make_chunk(ci):
                HRe = supers.tile([128, (NP + 1) * A * M], F32, tag="HRe")
                HIm = supers.tile([128, (NP + 1) * A * M], F32, tag="HIm")
                return {
                    "ci": ci,
                    "b0": ci * NB,
                    "HRe": HRe,
                    "HIm": HIm,
                    "ibase": (ci % 2) * 2 * A * M,
                }

            def row(T, j, i):
                return T[:, _off(j, i) : _off(j, i) + M]

            def rows3(T, j, i0, n):
                base = _off(j, i0)
                return T[:, base : base + n * M].rearrange("p (r c) -> p r c", r=n)

            def bc(ap, n):
                return ap[:, None, :].broadcast_to([128, n, M])

            def emit_load(C):
                # Plane order matches consumption: DVE needs 0,1,2 first,
                # Pool's first groups touch 6,7 then 8(y).
                b0 = C["b0"]
                for j in (0, 1, 6, 7, 2, 8, 3, 4, 5):
                    for comp in range(2):
                        if j == 8:
                            src = (ydre, ydim)[comp][:, b0 : b0 + NB]
                        else:
                            src = (hdre, hdim)[comp][j, :, b0 : b0 + NB]
                        stage = stg.tile([M, A * FS], F32, tag="stage")
                        nc.sync.dma_start(stage, src)
                        for ig in range(2):
                            ps = psin.tile([128, 4 * M], F32, tag="psin")
                            for q in range(4):
                                i = ig * 4 + q
                                nc.tensor.transpose(
                                    ps[:, q * M : (q + 1) * M],
                                    stage[:, i * FS : (i + 1) * FS],
                                    ident[:M, :M],
                                )
                            base = _off(j, ig * 4)
                            nc.scalar.copy(
                                C[("HRe", "HIm")[comp]][:, base : base + 4 * M],
                                ps,
                            )

            def inv_pair(C, k, n=None):
                # (ir_k, ii_k) as [128, 2, M]; broadcast over n rows if set
                b = C["ibase"]
                v = INV[:, b + k * M : b + k * M + 2 * A * M].rearrange(
                    "p (j c) -> p j c", j=2
                )[:, :, :M]
                if n is None:
                    return v
                return v[:, :, None, :].broadcast_to([128, 2, n, M])

            def pivot_chain(C, k):
                # ir_k + i*ii_k = conj(p)/|p|^2 for pivot p of step k
                b = C["ibase"]
                a = row(C["HRe"], k, k)
                b_ = row(C["HIm"], k, k)
                nc.vector.tensor_mul(TD, a, a)
                nc.vector.tensor_mul(TU, b_, b_)
                nc.vector.tensor_add(TD, TD, TU)
                nc.vector.reciprocal_approx_fast(TR, TD)
                irk = INV[:, b + k * M : b + (k + 1) * M]
                iik = INV[:, b + (A + k) * M : b + (A + k + 1) * M]
                nc.vector.tensor_mul(irk, a, TR)
                nc.vector.tensor_mul(iik, b_, TR)

            def factors(C, k):
                # F = H[i,k] * conj(p)/|p|^2 (unnegated) for i in k+1..7
                n = A - 1 - k
                car = rows3(C["HRe"], k, k + 1, n)
                cai = rows3(C["HIm"], k, k + 1, n)
                car4 = car[:, None, :, :].broadcast_to([128, 2, n, M])
                cai4 = cai[:, None, :, :].broadcast_to([128, 2, n, M])
                p1 = PAs[:, : 2 * n * M].rearrange("p (j c) -> p j c", j=2)
                p2 = PBs[:, : 2 * n * M].rearrange("p (j c) -> p j c", j=2)
                nc.vector.tensor_mul(p1, car4, inv_pair(C, k, n))
                nc.vector.tensor_mul(p2, cai4, inv_pair(C, k, n))
                fre = FRe[k % 2][:, : n * M]
                fim = FIm[k % 2][:, : n * M]
                # fre = cr*ir + ci*ii, fim = ci*ir - cr*ii
                nc.vector.tensor_add(fre, PAs[:, : n * M], PBs[:, n * M : 2 * n * M])
                nc.vector.tensor_sub(fim, PBs[:, : n * M], PAs[:, n * M : 2 * n * M])

            def elim_group(C, eng, sa, sb, k, j0, w, i0, nr):
                """Eliminate col k from planes [j0, j0+w), rows [i0, i0+nr).

                H[i,j] -= F_i * H[k,j]:  hr -= fre*Br - fim*Bi,
                                         hi -= fre*Bi + fim*Br.
                """
                HRe_, HIm_ = C["HRe"], C["HIm"]

                def wrows(T):
                    base = _off(j0, i0)
                    return T[:, base : base + w * A * M].rearrange(
                        "p (w c) -> p w c", w=w
                    )[:, :, : nr * M]

                def wrow_b(T):
                    base = _off(j0, k)
                    v = T[:, base : base + w * A * M].rearrange(
                        "p (w c) -> p w c", w=w
                    )[:, :, :M]
                    return v[:, :, None, :].broadcast_to([128, w, nr, M])

                def fw(Ft):
                    o = (i0 - k - 1) * M
                    v = Ft[:, o : o + nr * M].rearrange("p (r c) -> p r c", r=nr)
                    return v[:, None, :, :].broadcast_to([128, w, nr, M])

                hr, hi = wrows(HRe_), wrows(HIm_)
                Br, Bi = wrow_b(HRe_), wrow_b(HIm_)
                frew, fimw = fw(FRe[k % 2]), fw(FIm[k % 2])
                sz = w * nr * M
                A4 = sa[:, :sz].rearrange("p (w r c) -> p w r c", w=w, r=nr)
                A3 = sa[:, :sz].rearrange("p (w c) -> p w c", w=w)
                B4 = sb[:, :sz].rearrange("p (w r c) -> p w r c", w=w, r=nr)
                B3 = sb[:, :sz].rearrange("p (w c) -> p w c", w=w)
                eng.tensor_mul(A4, frew, Br)
                eng.tensor_mul(B4, frew, Bi)
                eng.tensor_sub(hr, hr, A3)
                eng.tensor_sub(hi, hi, B3)
                eng.tensor_mul(A4, fimw, Bi)
                eng.tensor_mul(B4, fimw, Br)
                eng.tensor_add(hr, hr, A3)
                eng.tensor_sub(hi, hi, B3)

            def emit_groups(C, eng, sa, sb, k, planes, i0, nr, wmax):
                js = list(planes)
                while js:
                    w = 1
                    while w < wmax and w < len(js) and js[w] == js[0] + w:
                        w += 1
                    elim_group(C, eng, sa, sb, k, js[0], w, i0, nr)
                    js = js[w:]

            def emit_prep0(C):
                pivot_chain(C, 0)
                factors(C, 0)

            def fwd_step(C, k, dve=True, pool=True, bias=0.0):
                """Elimination step k.  Emits the Pool share and/or the DVE
                share (incl. next step's pivot+factors pipelining)."""
                n = A - 1 - k
                npl, rs = plan_fwd(k, bias)
                j_b = 8 - npl
                if dve:
                    # pivot plane k+1 first, then next step's prep
                    elim_group(C, nc.vector, PAs, PBs, k, k + 1, 1, k + 1, n)
                    pivot_chain(C, k + 1)
                    if k + 1 < A - 1:
                        factors(C, k + 1)
                if pool:
                    pool_planes = list(range(j_b + 1, 9))
                    if pool_planes:
                        emit_groups(
                            C, nc.gpsimd, PPa, PPb, k, pool_planes, k + 1, n, W_P
                        )
                    if rs > 0:
                        elim_group(C, nc.gpsimd, PPa, PPb, k, j_b, 1, 8 - rs, rs)
                if dve:
                    dve_full = [j for j in range(k + 2, j_b + (0 if rs else 1))]
                    if dve_full:
                        emit_groups(
                            C, nc.vector, PAs, PBs, k, dve_full, k + 1, n, W_D
                        )
                    if rs > 0 and (n - rs) > 0:
                        elim_group(C, nc.vector, PAs, PBs, k, j_b, 1, k + 1, n - rs)

            def bwd_step(C, k, use_pool):
                """Jordan back step k: x_k = y_k*conj(p)/|p|^2, store it,
                then clear column k above the diagonal."""
                HRe_, HIm_ = C["HRe"], C["HIm"]
                yr = row(HRe_, 8, k)
                yi = row(HIm_, 8, k)
                p1 = PAs[:, : 2 * M].rearrange("p (j c) -> p j c", j=2)
                p2 = PBs[:, : 2 * M].rearrange("p (j c) -> p j c", j=2)
                yr2 = yr[:, None, :].broadcast_to([128, 2, M])
                yi2 = yi[:, None, :].broadcast_to([128, 2, M])
                nc.vector.tensor_mul(p1, yr2, inv_pair(C, k))
                nc.vector.tensor_mul(p2, yi2, inv_pair(C, k))
                # xr = yr*ir + yi*ii, xi = yi*ir - yr*ii
                nc.vector.tensor_add(yr, PAs[:, :M], PBs[:, M : 2 * M])
                nc.vector.tensor_sub(yi, PBs[:, :M], PAs[:, M : 2 * M])
                # x_k is final now -- store it while the back pass continues.
                so = stgo.tile([M, 2 * FS], F32, tag="so")
                so3 = so.rearrange("p (f c) -> p f c", c=2)
                for comp in range(2):
                    po = pso_pool.tile([M, FS], F32, tag="pso")
                    nc.tensor.transpose(
                        po, row(C[("HRe", "HIm")[comp]], 8, k), ident[:128, :128]
                    )
                    nc.scalar.copy(so3[:, :, comp], po)
                nc.sync.dma_start(out[k, :, C["b0"] : C["b0"] + NB], so)
                if k == 0:
                    return
                m = plan_bwd(k) if use_pool else 0

                def yupd(eng, sa, sb, r0, nr):
                    # y_i -= H[i,k]*x_k for rows [r0, r0+nr)
                    cr = rows3(HRe_, k, r0, nr)
                    ci_ = rows3(HIm_, k, r0, nr)
                    xrB = bc(yr, nr)
                    xiB = bc(yi, nr)
                    qa = sa[:, : nr * M].rearrange("p (r c) -> p r c", r=nr)
                    qc = sa[:, 7 * M : (7 + nr) * M].rearrange(
                        "p (r c) -> p r c", r=nr
                    )
                    qb = sb[:, : nr * M].rearrange("p (r c) -> p r c", r=nr)
                    qd = sb[:, 7 * M : (7 + nr) * M].rearrange(
                        "p (r c) -> p r c", r=nr
                    )
                    ytr = rows3(HRe_, 8, r0, nr)
                    yti = rows3(HIm_, 8, r0, nr)
                    eng.tensor_mul(qa, cr, xrB)
                    eng.tensor_mul(qc, cr, xiB)
                    eng.tensor_sub(ytr, ytr, qa)
                    eng.tensor_sub(yti, yti, qc)
                    eng.tensor_mul(qb, ci_, xiB)
                    eng.tensor_mul(qd, ci_, xrB)
                    eng.tensor_add(ytr, ytr, qb)
                    eng.tensor_sub(yti, yti, qd)

                if m > 0:
                    yupd(nc.gpsimd, PPa, PPb, 0, m)
                if k - m > 0:
                    yupd(nc.vector, PAs, PBs, m, k - m)

            # ---------------- emission schedule ----------------
            c0 = make_chunk(0)
            c1 = make_chunk(1)

            emit_load(c0)
            emit_prep0(c0)
            for k in range(A - 1):
                fwd_step(c0, k)
            emit_load(c1)
            # chunk 1 prep + Pool's step-0 share start while DVE runs
            # chunk 0's back pass, interleaved with chunk 1's DVE forward
            # steps; the planner bias hands Pool the slack.
            emit_prep0(c1)
            fwd_step(c1, 0, dve=False, pool=True, bias=bwd_step_cost(7))
            bwd_step(c0, 7, use_pool=False)
            fwd_step(c1, 0, dve=True, pool=False, bias=bwd_step_cost(7))
            for k in range(1, A - 1):
                j = 7 - k
                bwd_step(c0, j, use_pool=False)
                fwd_step(c1, k, bias=bwd_step_cost(j))
            bwd_step(c0, 0, use_pool=False)
            for k in range(A - 1, -1, -1):
                bwd_step(c1, k, use_pool=True)

    nc.finalize()
    return nc


_NC_CACHE = None


def _get_nc():
    global _NC_CACHE
    if _NC_CACHE is None:
        _NC_CACHE = _build()
    return _NC_CACHE


def _prep_core(y_re, y_im, h_re, h_im, c):
    """Host-side shard prep for core c: f-slice + block-diagonal extraction."""
    fsl = slice(c * FS, (c + 1) * FS)
    ue = np.arange(U)
    maps = {}
    for name, h in (("hd_re", h_re), ("hd_im", h_im)):
        h6 = h[:, 0, :, :, :, :, fsl].reshape(B, U, A, U, A, S, FS)
        hd = h6[:, ue, :, ue]              # [u, b, i, j, s, f]
        maps[name] = np.ascontiguousarray(
            hd.transpose(3, 0, 1, 4, 2, 5), dtype=np.float32
        )                                   # [j, u, b, s, i, f]
    for name, y in (("yd_re", y_re), ("yd_im", y_im)):
        y5 = y[:, 0, :, :, fsl].reshape(B, U, A, S, FS)   # [b, u, i, s, f]
        maps[name] = np.ascontiguousarray(
            y5.transpose(1, 0, 3, 2, 4), dtype=np.float32
        )                                   # [u, b, s, i, f]
    return maps


def kernel(y_re, y_im, h_re, h_im, **_ignored):
    global LAST_RESULTS
    y_re = np.asarray(y_re, dtype=np.float32)
    y_im = np.asarray(y_im, dtype=np.float32)
    h_re = np.asarray(h_re, dtype=np.float32)
    h_im = np.asarray(h_im, dtype=np.float32)

    nc = _get_nc()
    in_maps = [_prep_core(y_re, y_im, h_re, h_im, c) for c in range(NCORES)]
    trace = bool(int(os.environ.get("BD_TRACE", "0")))
    res = run_bass_kernel_spmd(
        nc, in_maps, core_ids=list(range(NCORES)), trace=trace
    )
    LAST_RESULTS = res
    outs = []
    for r in res.results:
        o = r["out"]                              # [i, u, b, s, f, c]
        o = o.transpose(2, 1, 0, 3, 4, 5)         # [b, u, i, s, f, c]
        outs.append(o.reshape(B, NR, S, FS, 2))
    full = np.concatenate(outs, axis=3)           # [B, NR, S, F, 2]
    return np.ascontiguousarray(full[:, None])    # [B, 1, NR, S, F, 2]
